# revision 8
# baseline (speedup 1.0000x reference)
"""Trainium2 Bass kernel for nn_MCLoss (scatter_memory forward).

Computes logits = inputs @ memory.T  ([4096, 2048] @ [2048, 50000] -> [4096, 50000] f32).

Strategy (tensor-parallel, per sharding hint): the memory bank is sharded
row-wise across 8 NeuronCores (exactly 6250 identity rows each, no padding).
Each core computes its [4096, 6250] slice of the logits with a tiled PE
matmul; the host concatenates the 8 slices.

Device kernel (per core, identical SPMD program):
  - Operands in bf16 (PE runs 1 cycle/row, same as fp32r, but half the DMA
    bytes and SBUF footprint); logits stored bf16 and upcast on host.
    Total rel err ~2.6e-3 on unit-norm rows vs the 2e-2 gate.
  - lhs (stationary tiles): inputs pre-transposed on host into
    [128, 32, 16, 128] tile layout. A[p, m, k, j] = inputs[m*128 + j, k*128 + p].
    All 32 m-tiles (16.8 MB) are DMA'd once and stay RESIDENT in SBUF for the
    whole kernel - no re-loads.
  - rhs (moving operand): memory shard transposed on host to [2048, 6250]
    (memT[d, c] = memory[c, d]) bf16, streamed per column group.
  - Column groups: 2x309 + 11x512 = 6250 exactly (all widths >= 256 so every
    matmul runs at the full 1 cycle/row rate; each group's PSUM tile is one
    bank so no matmul crosses a bank boundary; zero padding waste).
  - Loop nest keeps the PE dense: for each group, 32 m-tiles x 16 accumulating
    k-matmuls back-to-back; PSUM evicted via VectorE to SBUF and DMA'd out on
    the ACT HWDGE ring (input loads use the SP ring) so stores never
    head-of-line-block loads.
  - Measured at the PE row-streaming floor: 32m x 16k x 6250 = 3.2M moving
    rows/core; per-rep device time tracks rows/clock exactly (CoreSim 1.333ms
    at 2.4 GHz; ~1.64ms on HW under the sustained-load P0 clock ~1.95 GHz).
    A variant with memory stationary and 8.5x fewer weight loads measured
    identical per-row time, confirming no LDWEIGHTS exposure remains.

kernel._build(reps=N) emits the compute body N times (idempotent writes) so
test.py can measure marginal per-rep device time with the axon per-dispatch
overhead cancelled.
"""
import numpy as np
import ml_dtypes

import concourse.bass as bass
import concourse.mybir as mybir
import concourse.tile as tile
from concourse import bacc
from concourse.bass_utils import run_bass_kernel_spmd

P = 128
B = 4096          # rows of inputs
D = 2048          # features (contraction)
C = 50000         # memory rows (classes)
N_CORES = 8
N_SHARD = C // N_CORES          # 6250 per-core logits width (exact)
MT = B // P                     # 32 m-tiles
KT = D // P                     # 16 k-tiles

# Column groups: two 309-wide first (cheap first tile -> earlier PE start),
# then eleven 512-wide. 2*309 + 11*512 = 6250.
GROUPS = []
_c0 = 0
for _w in [309, 309] + [512] * 11:
    GROUPS.append((_c0, _w))
    _c0 += _w
assert _c0 == N_SHARD

_NC_CACHE = {}


def _build(reps=1):
    """Build the SPMD program. reps>1 repeats the whole compute body (same
    inputs -> same outputs, idempotent) so test.py can measure the marginal
    per-rep device time with launch overhead cancelled out."""
    if reps in _NC_CACHE:
        return _NC_CACHE[reps]
    dt_in = mybir.dt.bfloat16
    nc = bacc.Bacc("TRN2", target_bir_lowering=False, debug=False)
    lhs = nc.dram_tensor("lhs", [P, MT, KT, P], dt_in, kind="ExternalInput")
    rhs = nc.dram_tensor("rhs", [D, N_SHARD], dt_in, kind="ExternalInput")
    # Logits stored as bf16 (halves output HBM traffic; adds ~1.1e-3 rel err,
    # total ~2.3e-3 vs the 2e-2 gate). Host upcasts to f32.
    out = nc.dram_tensor("out", [B, N_SHARD], mybir.dt.bfloat16, kind="ExternalOutput")
    rhs_r = rhs[:].rearrange("(k p) c -> p k c", p=P)

    with tile.TileContext(nc) as tc:
        with (
            tc.tile_pool(name="rhsp", bufs=2) as rhsp,
            tc.tile_pool(name="lhsp", bufs=MT) as lhsp,
            tc.tile_pool(name="outp", bufs=4) as outp,
            tc.tile_pool(name="psump", bufs=6, space="PSUM") as psump,
        ):
            # First group's rhs, then the whole lhs (resident for the kernel).
            c0_0, w0 = GROUPS[0]
            rt = rhsp.tile([P, KT, w0], dt_in, tag="rhs")
            nc.sync.dma_start(out=rt[:], in_=rhs_r[:, :, c0_0 : c0_0 + w0])
            lts = []
            for m in range(MT):
                lt = lhsp.tile([P, KT, P], dt_in, tag="lhs")
                nc.sync.dma_start(out=lt[:], in_=lhs[:, m, :, :])
                lts.append(lt)

            for rep in range(reps):
                for gi, (c0, w) in enumerate(GROUPS):
                    if gi > 0 or rep > 0:
                        rt = rhsp.tile([P, KT, w], dt_in, tag="rhs")
                        nc.sync.dma_start(out=rt[:], in_=rhs_r[:, :, c0 : c0 + w])
                    for m in range(MT):
                        ps = psump.tile([P, w], mybir.dt.float32, tag="ps")
                        for k in range(KT):
                            nc.tensor.matmul(
                                ps[:],
                                lhsT=lts[m][:, k, :],
                                rhs=rt[:, k, :],
                                start=(k == 0),
                                stop=(k == KT - 1),
                            )
                        ot = outp.tile([P, w], mybir.dt.bfloat16, tag="out")
                        nc.vector.tensor_copy(out=ot[:], in_=ps[:])
                        nc.scalar.dma_start(
                            out=out[m * P : (m + 1) * P, c0 : c0 + w], in_=ot[:]
                        )
    nc.compile()
    _NC_CACHE[reps] = nc
    return nc


def _prep_inputs(inputs, memory):
    inputs = np.ascontiguousarray(np.asarray(inputs, dtype=np.float32))
    # lhs tile layout: A[p, m, k, j] = inputs[m*128 + j, k*128 + p], bf16
    lhs_np = np.ascontiguousarray(
        inputs.reshape(MT, P, KT, P).transpose(3, 0, 2, 1).astype(ml_dtypes.bfloat16)
    )
    # per-core rhs: memT shard [D, 6250] bf16 (cast first, then transpose views)
    mem_bf = np.asarray(memory, dtype=np.float32).astype(ml_dtypes.bfloat16)
    rhs_all = np.ascontiguousarray(
        mem_bf.reshape(N_CORES, N_SHARD, D).transpose(0, 2, 1)
    )
    return lhs_np, rhs_all


def kernel(inputs, targets, memory):
    """Full-input entry point: returns logits [4096, 50000] float32."""
    nc = _build()
    lhs_np, rhs_all = _prep_inputs(inputs, memory)
    in_maps = [{"lhs": lhs_np, "rhs": rhs_all[c]} for c in range(N_CORES)]
    res = run_bass_kernel_spmd(nc, in_maps, core_ids=list(range(N_CORES)))
    logits = np.concatenate(
        [res.results[c]["out"].astype(np.float32) for c in range(N_CORES)], axis=1
    )
    return np.ascontiguousarray(logits)
